# revision 25
# baseline (speedup 1.0000x reference)
"""ConvLSTM cell forward on 8 Trainium2 NeuronCores — Winograd F(2,3).

Problem: B=16, Cin=64, Chid=128, H=W=64, K=3 (SAME padding).
  ig = sigmoid(conv(x,Wxi) + bxi + conv(h,Whi) + Wci*c)
  fg = sigmoid(conv(x,Wxf) + bxf + conv(h,Whf) + Wcf*c)
  c_new = fg*c + ig*tanh(conv(x,Wxc) + bxc + conv(h,Whc))
  og = sigmoid(conv(x,Wxo) + bxo + conv(h,Who) + Wco*c)
  h_new = og*c_new
  returns (og, h_new, c_new)

Strategy:
  - Data-parallel over batch: 2 images per core, weights replicated.
  - Winograd F(2,3) along W cuts the 3-tap x-dim contraction from 3
    multiplies per output to 4 per 2 outputs. Per output-column pair
    (2t, 2t+1) with padded input cols d_k = p[2t+k]:
      V0 = d0-d2, V1 = d1+d2, V2 = d2-d1, V3 = d1-d3      (input xform)
      M_j = sum_dy U_j(dy) @ V_j(row+dy)                   (matmuls)
      out[2t] = M0+M1+M2,  out[2t+1] = M1-M2-M3            (output xform)
    U_j are host-transformed weights: u0=g0, u1=(g0+g1+g2)/2,
    u2=(g0-g1+g2)/2, u3=g2.
  - The 4 V input planes are precomputed on the host (prep is free) and
    DMA'd in directly; no on-device input transform at all.
  - h convs: per gate 4 j-planes x 3 dy = 12 K=128 matmuls per chunk.
  - x convs: the V^x planes pack [rows r ; rows r+1] on 128 partitions,
    so one matmul covers dy=0+1; dy=2 is a K=64 matmul on the lower
    half: 8 x-matmuls per gate per chunk. 20 total (vs 28 direct).
  - Chunk = 16 output rows: M_j is [128, 512] = one full PSUM bank;
    4 banks per gate, 2 gates in flight (8 banks).
  - Output transform on DVE with one-PSUM-operand chains (HW limit):
    even: e1=M1+pe_e; e2=e1+M2; M0=e2+M0; sigmoid reads the M0 bank.
    odd:  o1=M1+pe_o; o2=o1-M2; o3=o2-M3; sigmoid reads SBUF o3.
  - Peephole products (Wc*c) are precomputed on the host in the eo
    layout and DMA'd in; bias rides the activation instruction.
  - og/c_new/h_new are assembled pixel-interleaved in SBUF (strided dst
    APs are free for 1x-rate ops) so output DMAs are contiguous fp32.
  - Matmuls in fp16, accumulate fp32; elementwise fp16 where possible;
    t1/t2/c_new/h_new on GpSimd.
"""

import os
import numpy as np

B, CIN, CHID, H, W, K = 16, 64, 128, 64, 64, 3
N_CORES = 8
PER = B // N_CORES          # images per core
WPAD = W + 2                # 66 padded cols = 33 even + 33 odd
RPAD = H + 2                # 66 padded rows
NT = W // 2                 # 32 tiles per row
NJ = 4                      # winograd planes
VFLAT = RPAD * NT           # 2112 = V plane cols per channel
CHUNK_ROWS = 16
CCT = CHUNK_ROWS * NT       # 512 tile-cols per chunk = one PSUM bank
CCP = CHUNK_ROWS * W        # 1024 pixels per chunk
N_CHUNKS = H // CHUNK_ROWS  # 4
HW = H * W

# gate processing order: candidate first (tanh can start early), output gate
# last (shortest tail after the final matmul). Index meaning: 0=i 1=f 2=o 3=c
GORDER = [3, 0, 1, 2]
# V-plane DMA row pieces
VPIECES = [(0, 34), (34, 66)]
# output row chunks per image: the last image ends with two 8-row chunks so
# the post-matmul tail (combines/sigmoid/h_new/DMA) is halved
CHUNKS_MID = [(0, 16), (16, 16), (32, 16), (48, 16)]
CHUNKS_LAST = [(0, 16), (16, 16), (32, 16), (48, 8), (56, 8)]

_PROG = None
LAST_RESULTS = None


def _build_program():
    import concourse.bacc as bacc
    import concourse.tile as tile
    import concourse.mybir as mybir
    from contextlib import ExitStack

    f32 = mybir.dt.float32
    f16 = mybir.dt.float16

    nc = bacc.Bacc("TRN2", target_bir_lowering=False, debug=False,
                   num_devices=N_CORES)

    vh_d = nc.dram_tensor("vh", [PER, NJ, CHID, VFLAT], f16,
                          kind="ExternalInput").ap()
    vx_d = nc.dram_tensor("vx", [PER, NJ, 2 * CIN, VFLAT], f16,
                          kind="ExternalInput").ap()
    c_d = nc.dram_tensor("c", [PER, CHID, HW], f16, kind="ExternalInput").ap()
    pe_d = nc.dram_tensor("pe", [3, PER, CHID, HW], f16,
                          kind="ExternalInput").ap()
    whw_d = nc.dram_tensor("whw", [4, CHID, 12 * CHID], f16,
                           kind="ExternalInput").ap()
    wxw_d = nc.dram_tensor("wxw", [4, CHID, 4 * CHID], f16,
                           kind="ExternalInput").ap()
    wx2_d = nc.dram_tensor("wx2", [4, CHID, 4 * CHID], f16,
                           kind="ExternalInput").ap()
    bias_d = nc.dram_tensor("bias", [CHID, 4], f32, kind="ExternalInput").ap()
    og_d = nc.dram_tensor("og", [PER, CHID, HW], f32, kind="ExternalOutput").ap()
    hn_d = nc.dram_tensor("hn", [PER, CHID, HW], f32, kind="ExternalOutput").ap()
    cn_d = nc.dram_tensor("cn", [PER, CHID, HW], f32, kind="ExternalOutput").ap()

    SIG = mybir.ActivationFunctionType.Sigmoid
    TANH = mybir.ActivationFunctionType.Tanh

    with tile.TileContext(nc) as tc, ExitStack() as ctx:
        const = ctx.enter_context(tc.tile_pool(name="const", bufs=1))
        vpool = ctx.enter_context(tc.tile_pool(name="vpool", bufs=2))
        work = ctx.enter_context(tc.tile_pool(name="work", bufs=2))
        outs = ctx.enter_context(tc.tile_pool(name="outs", bufs=2))
        psum = ctx.enter_context(tc.tile_pool(name="psum", bufs=8, space="PSUM"))

        # ---- weights on the Activation HWDGE queue, consumption order.
        # Gate 3 (candidate, processed first) is issued before the first
        # image's V^x pieces; the remaining gates after them.
        whw_t, wxw_t, wx2_t = {}, {}, {}

        def walloc(g):
            whw_t[g] = [const.tile([CHID, 6 * CHID], f16, name=f"whw{g}_{p}")
                        for p in range(2)]
            wxw_t[g] = const.tile([CHID, 4 * CHID], f16, name=f"wxw{g}")
            wx2_t[g] = const.tile([CHID, 4 * CHID], f16, name=f"wx2{g}")

        def wdma(g):
            nc.scalar.dma_start(whw_t[g][0][:], whw_d[g][:, 0:6 * CHID])
            nc.scalar.dma_start(wxw_t[g][:], wxw_d[g])
            nc.scalar.dma_start(wx2_t[g][:], wx2_d[g])
            nc.scalar.dma_start(whw_t[g][1][:],
                                whw_d[g][:, 6 * CHID:12 * CHID])

        for g in GORDER:
            walloc(g)
        bias_t = const.tile([CHID, 4], f32)
        wdma(GORDER[0])
        nc.scalar.dma_start(bias_t[:], bias_d)

        def uh(g, j, dy):
            p, blk = divmod(j * 3 + dy, 6)
            return whw_t[g][p][:, blk * CHID:(blk + 1) * CHID]

        for b in range(PER):
            chunks = CHUNKS_LAST if b == PER - 1 else CHUNKS_MID
            # V planes: vh rides the SP queue, vx the Activation queue
            # (interleaved with the weight tiles at startup)
            vh = [vpool.tile([CHID, VFLAT], f16, tag=f"vh{j}",
                             name=f"vh{b}_{j}") for j in range(NJ)]
            vx = [vpool.tile([2 * CIN, VFLAT], f16, tag=f"vx{j}",
                             name=f"vx{b}_{j}") for j in range(NJ)]

            def vdma(piece):
                r0, r1 = VPIECES[piece]
                s, e = r0 * NT, r1 * NT
                for j in range(NJ):
                    nc.sync.dma_start(vh[j][:, s:e], vh_d[b][j][:, s:e])
                    nc.scalar.dma_start(vx[j][:, s:e], vx_d[b][j][:, s:e])

            vdma(0)
            if b == 0:
                for g in GORDER[1:]:
                    wdma(g)

            for kc, (row0, nrows) in enumerate(chunks):
                cct = nrows * NT          # psum bank cols (tile space)
                ccp = nrows * W           # pixels in chunk
                c0 = row0 * W
                last = (b == PER - 1 and kc == len(chunks) - 1)
                ps = {}
                for g in GORDER:
                    m = [psum.tile([CHID, cct], f32, tag="ps",
                                   padded_shape=[CHID, CCT],
                                   name=f"ps{b}_{kc}_{g}_{j}")
                         for j in range(NJ)]
                    ps[g] = m
                    # final gate of the final chunk: do the M0 plane last so
                    # the odd-side chain drains during its matmuls
                    jorder = [1, 2, 3, 0] if (last and g == 2) else range(NJ)
                    for j in jorder:
                        for dy in range(3):
                            nc.tensor.matmul(
                                m[j][:], uh(g, j, dy),
                                vh[j][:, (row0 + dy) * NT:
                                       (row0 + dy) * NT + cct],
                                start=(dy == 0), stop=False)
                        nc.tensor.matmul(
                            m[j][:], wxw_t[g][:, j * CHID:(j + 1) * CHID],
                            vx[j][:, row0 * NT:row0 * NT + cct],
                            start=False, stop=False)
                        # dy=2: zero lower weight rows; the packed upper
                        # partitions at col offset row0+1 hold V^x(row+2)
                        nc.tensor.matmul(
                            m[j][:], wx2_t[g][:, j * CHID:(j + 1) * CHID],
                            vx[j][:, (row0 + 1) * NT:(row0 + 1) * NT + cct],
                            start=False, stop=True)

                # c and peephole chunks (host-precomputed pe = Wc*c), eo layout
                ctc = outs.tile([CHID, CCP], f16, tag="ct", bufs=3,
                                name=f"ct{b}_{kc}")
                nc.sync.dma_start(ctc[:, 0:ccp], c_d[b][:, c0:c0 + ccp])
                pec = []
                for gi in range(3):
                    t = outs.tile([CHID, CCP], f16, tag=f"pe{gi}", bufs=3,
                                  name=f"pe{b}_{kc}_{gi}")
                    nc.sync.dma_start(t[:, 0:ccp], pe_d[gi][b][:, c0:c0 + ccp])
                    pec.append(t)
                if kc == 0:
                    vdma(1)

                def v64(t):  # [128, ccp] -> [128, rows, 64]
                    return t[:, 0:ccp].rearrange("p (r s) -> p r s", s=64)

                def vt(pb):  # psum bank [128, cct] -> [128, rows, 32]
                    return pb[:].rearrange("p (r t) -> p r t", t=NT)

                # activations / gate results, [row][even 32 | odd 32] layout
                gc = work.tile([CHID, CCP], f16, tag="gc")
                ig = work.tile([CHID, CCP], f16, tag="ig")
                fg = work.tile([CHID, CCP], f16, tag="fg")
                # og/cn/hn assembled pixel-interleaved for contiguous DMA
                ogt = outs.tile([CHID, CCP], f32, tag="og")
                cnt = outs.tile([CHID, CCP], f32, tag="cn")
                hnt = outs.tile([CHID, CCP], f32, tag="hn")
                ogi = ogt[:, 0:ccp].rearrange("p (r t eo) -> p r t eo",
                                              eo=2, t=NT)

                for g in GORDER:
                    m = ps[g]
                    gi = g  # 0=i 1=f 2=o 3=c
                    if gi == 3:
                        # candidate chain stays fp32 end-to-end: tanh has
                        # unit slope so fp16 rounding would reach c_new
                        m1c = work.tile([CHID, CCT], f32, tag="m1c")
                        nc.scalar.activation(
                            m1c[:, 0:cct], m[1][:],
                            mybir.ActivationFunctionType.Copy)
                        e1 = work.tile([CHID, CCT], f32, tag="ce1")
                        nc.vector.tensor_add(e1[:, 0:cct], m1c[:, 0:cct],
                                             m[2][:])
                        nc.vector.tensor_add(m[0][:], e1[:, 0:cct], m[0][:])
                        o1 = work.tile([CHID, CCT], f32, tag="co1")
                        nc.vector.tensor_sub(o1[:, 0:cct], m1c[:, 0:cct],
                                             m[2][:])
                        o2 = work.tile([CHID, CCT], f32, tag="co2")
                        nc.vector.tensor_sub(o2[:, 0:cct], o1[:, 0:cct],
                                             m[3][:])
                        nc.scalar.activation(v64(gc)[:, :, 0:NT], vt(m[0]),
                                             TANH, bias=bias_t[:, 3:4])
                        nc.scalar.activation(v64(gc)[:, :, NT:W],
                                             o2[:, 0:cct].rearrange(
                                                 "p (r t) -> p r t", t=NT),
                                             TANH, bias=bias_t[:, 3:4])
                        continue
                    pv = v64(pec[gi])
                    # M1 is evacuated once by ScalarE; the pe+M1 partial
                    # sums then run on GpSimd (SBUF-only), leaving DVE just
                    # four one-PSUM-operand ops per gate
                    m1c = work.tile([CHID, CCT], f16, tag="m1g",
                                    name=f"m1g_{b}_{kc}_{gi}")
                    nc.scalar.activation(m1c[:, 0:cct], m[1][:],
                                         mybir.ActivationFunctionType.Copy)
                    qe = work.tile([CHID, CCT], f16, tag="qe",
                                   name=f"qe_{b}_{kc}_{gi}")
                    nc.gpsimd.tensor_add(vt(qe[:, 0:cct]), pv[:, :, 0:NT],
                                         vt(m1c[:, 0:cct]))
                    qo = work.tile([CHID, CCT], f16, tag="qo",
                                   name=f"qo_{b}_{kc}_{gi}")
                    nc.gpsimd.tensor_add(vt(qo[:, 0:cct]), pv[:, :, NT:W],
                                         vt(m1c[:, 0:cct]))
                    # even = (pe_e+M1)+M2+M0, finishing in the M0 bank
                    e2 = work.tile([CHID, CCT], f32, tag="e2",
                                   name=f"e2_{b}_{kc}_{gi}")
                    nc.vector.tensor_add(e2[:, 0:cct], qe[:, 0:cct],
                                         m[2][:])
                    nc.vector.tensor_add(m[0][:], e2[:, 0:cct], m[0][:])
                    # odd = (pe_o+M1)-M2-M3 in SBUF
                    o2 = work.tile([CHID, CCT], f32, tag="o2",
                                   name=f"o2_{b}_{kc}_{gi}")
                    nc.vector.tensor_sub(o2[:, 0:cct], qo[:, 0:cct],
                                         m[2][:])
                    o3 = work.tile([CHID, CCT], f32, tag="o3",
                                   name=f"o3_{b}_{kc}_{gi}")
                    nc.vector.tensor_sub(o3[:, 0:cct], o2[:, 0:cct],
                                         m[3][:])
                    if gi == 0:
                        dste, dsto = v64(ig)[:, :, 0:NT], v64(ig)[:, :, NT:W]
                    elif gi == 1:
                        dste, dsto = v64(fg)[:, :, 0:NT], v64(fg)[:, :, NT:W]
                    else:
                        dste, dsto = ogi[:, :, :, 0], ogi[:, :, :, 1]
                    nc.scalar.activation(dste, vt(m[0]), SIG,
                                         bias=bias_t[:, gi:gi + 1])
                    nc.scalar.activation(
                        dsto, o3[:, 0:cct].rearrange("p (r t) -> p r t",
                                                     t=NT),
                        SIG, bias=bias_t[:, gi:gi + 1])

                t2 = work.tile([CHID, CCP], f16, tag="t2")
                nc.gpsimd.tensor_mul(t2[:, 0:ccp], ig[:, 0:ccp],
                                     gc[:, 0:ccp])
                t1 = work.tile([CHID, CCP], f16, tag="t1")
                nc.gpsimd.tensor_mul(t1[:, 0:ccp], fg[:, 0:ccp],
                                     ctc[:, 0:ccp])
                cni = cnt[:, 0:ccp].rearrange("p (r t eo) -> p r eo t",
                                              eo=2, t=NT)
                src_eo = lambda t: t[:, 0:ccp].rearrange(
                    "p (r eo t) -> p r eo t", eo=2, t=NT)
                nc.gpsimd.tensor_add(cni, src_eo(t1), src_eo(t2))

                sl = slice(c0, c0 + ccp)
                nc.sync.dma_start(cn_d[b][:, sl], cnt[:, 0:ccp])
                # h_new split even/odd so the odd half rides right after
                # the last sigmoid; on DVE for the final chunk (shorter tail)
                heng = nc.vector if last else nc.gpsimd
                hni = hnt[:, 0:ccp].rearrange("p (r t eo) -> p r t eo",
                                              eo=2, t=NT)
                cnx = cnt[:, 0:ccp].rearrange("p (r t eo) -> p r t eo",
                                              eo=2, t=NT)
                heng.tensor_mul(hni[:, :, :, 0], ogi[:, :, :, 0],
                                cnx[:, :, :, 0])
                heng.tensor_mul(hni[:, :, :, 1], ogi[:, :, :, 1],
                                cnx[:, :, :, 1])
                nc.sync.dma_start(og_d[b][:, sl], ogt[:, 0:ccp])
                nc.sync.dma_start(hn_d[b][:, sl], hnt[:, 0:ccp])

    nc.compile()
    return nc


def _pad_eo(a, rowshift=0):
    """[N,C,H,W] fp32 -> even/odd col-split padded fp32 [N,C,RPAD,2,33]."""
    n, ch = a.shape[:2]
    p = np.zeros((n, ch, RPAD + rowshift, WPAD), np.float32)
    p[:, :, 1:H + 1, 1:W + 1] = a
    p = p[:, :, rowshift:rowshift + RPAD]
    return np.stack([p[:, :, :, 0::2], p[:, :, :, 1::2]], axis=3)


def _v_planes(eo):
    """eo: [N,C,RPAD,2,33] -> [N, 4, C, RPAD*NT] fp16 Winograd planes."""
    he, ho = eo[:, :, :, 0], eo[:, :, :, 1]
    v = np.stack([he[..., 0:NT] - he[..., 1:NT + 1],
                  ho[..., 0:NT] + he[..., 1:NT + 1],
                  he[..., 1:NT + 1] - ho[..., 0:NT],
                  ho[..., 0:NT] - ho[..., 1:NT + 1]], axis=1)
    n, _, ch = v.shape[:3]
    return np.ascontiguousarray(v.reshape(n, NJ, ch, RPAD * NT)
                                ).astype(np.float16)


def _eo_pixels(a):
    """[N,C,H,W] fp32 -> [N,C,HW] fp16 with each row as [even 32 | odd 32]."""
    eo = np.concatenate([a[:, :, :, 0::2], a[:, :, :, 1::2]], axis=3)
    return np.ascontiguousarray(eo.reshape(a.shape[0], a.shape[1], HW)
                                ).astype(np.float16)


def _wino_u(g):
    """g: [..., 3] -> [..., 4] Winograd F(2,3) weight transform."""
    u0 = g[..., 0]
    u1 = 0.5 * (g[..., 0] + g[..., 1] + g[..., 2])
    u2 = 0.5 * (g[..., 0] - g[..., 1] + g[..., 2])
    u3 = g[..., 2]
    return np.stack([u0, u1, u2, u3], axis=-1)


def kernel(x, h, c, Wxi, bxi, Whi, Wci, Wxf, bxf, Whf, Wcf,
           Wxo, bxo, Who, Wco, Wxc, bxc, Whc):
    global _PROG, LAST_RESULTS
    from concourse.bass_utils import run_bass_kernel_spmd

    x = np.asarray(x, dtype=np.float32)
    h = np.asarray(h, dtype=np.float32)
    c = np.asarray(c, dtype=np.float32)

    vh = _v_planes(_pad_eo(h))
    # x: [x_pad rows r ; x_pad rows r+1] packed on the channel axis
    xeo = np.concatenate([_pad_eo(x), _pad_eo(x, rowshift=1)], axis=1)
    vx = _v_planes(xeo)
    cf = _eo_pixels(c)
    pe = np.stack([_eo_pixels(np.asarray(wc, np.float32)[None] * c)
                   for wc in (Wci, Wcf, Wco)])  # [3, B, 128, HW] fp16

    def wh_prep(w):
        # [Co=128, Ci=128, 3, 3] -> U [Ci, (j*3+dy)*128 + Co]
        u = _wino_u(np.asarray(w, np.float32))  # [co, ci, dy, j]
        out = np.empty((CHID, 12 * CHID), np.float32)
        for j in range(NJ):
            for dy in range(3):
                out[:, (j * 3 + dy) * CHID:(j * 3 + dy + 1) * CHID] = \
                    u[:, :, dy, j].T
        return out.astype(np.float16)

    def wx_prep(w):
        # [Co=128, Ci=64, 3, 3] -> (packed dy0/dy1 [128, 4*128],
        # dy2 [128, 4*128] with zero lower rows — rhs is the packed vx at
        # row offset +1, whose upper partitions hold V^x(row+2))
        u = _wino_u(np.asarray(w, np.float32))  # [co, ci, dy, j]
        p01 = np.empty((CHID, 4 * CHID), np.float32)
        p2 = np.zeros((CHID, 4 * CHID), np.float32)
        for j in range(NJ):
            p01[:CIN, j * CHID:(j + 1) * CHID] = u[:, :, 0, j].T
            p01[CIN:, j * CHID:(j + 1) * CHID] = u[:, :, 1, j].T
            p2[CIN:, j * CHID:(j + 1) * CHID] = u[:, :, 2, j].T
        return p01.astype(np.float16), p2.astype(np.float16)

    whw = np.stack([wh_prep(w) for w in (Whi, Whf, Who, Whc)])
    wxp = [wx_prep(w) for w in (Wxi, Wxf, Wxo, Wxc)]
    wxw = np.stack([p for p, _ in wxp])
    wx2 = np.stack([q for _, q in wxp])
    bias = np.ascontiguousarray(np.stack(
        [np.asarray(v, dtype=np.float32) for v in (bxi, bxf, bxo, bxc)], axis=1))

    if _PROG is None:
        _PROG = _build_program()

    in_maps = []
    for i in range(N_CORES):
        sl = slice(i * PER, (i + 1) * PER)
        in_maps.append({
            "vh": np.ascontiguousarray(vh[sl]),
            "vx": np.ascontiguousarray(vx[sl]),
            "c": np.ascontiguousarray(cf[sl]),
            "pe": np.ascontiguousarray(pe[:, sl]),
            "whw": whw, "wxw": wxw, "wx2": wx2, "bias": bias,
        })

    res = run_bass_kernel_spmd(nc=_PROG, in_maps=in_maps,
                               core_ids=list(range(N_CORES)),
                               trace=bool(os.environ.get("KERNEL_TRACE")))
    LAST_RESULTS = res

    og = np.empty((B, CHID, HW), dtype=np.float32)
    hn = np.empty((B, CHID, HW), dtype=np.float32)
    cn = np.empty((B, CHID, HW), dtype=np.float32)
    for i in range(N_CORES):
        sl = slice(i * PER, (i + 1) * PER)
        og[sl] = res.results[i]["og"]
        hn[sl] = res.results[i]["hn"]
        cn[sl] = res.results[i]["cn"]

    shape = (B, CHID, H, W)
    return (og.reshape(shape), hn.reshape(shape), cn.reshape(shape))


# revision 30
# speedup vs baseline: 1.1139x; 1.1139x over previous
"""ConvLSTM cell forward on 8 Trainium2 NeuronCores — Winograd F(2,3).

Problem: B=16, Cin=64, Chid=128, H=W=64, K=3 (SAME padding).
  ig = sigmoid(conv(x,Wxi) + bxi + conv(h,Whi) + Wci*c)
  fg = sigmoid(conv(x,Wxf) + bxf + conv(h,Whf) + Wcf*c)
  c_new = fg*c + ig*tanh(conv(x,Wxc) + bxc + conv(h,Whc))
  og = sigmoid(conv(x,Wxo) + bxo + conv(h,Who) + Wco*c)
  h_new = og*c_new
  returns (og, h_new, c_new)

Strategy:
  - Data-parallel over batch: 2 images per core, weights replicated.
  - Winograd F(2,3) along W cuts the 3-tap x-dim contraction from 3
    multiplies per output to 4 per 2 outputs. Per output-column pair
    (2t, 2t+1) with padded input cols d_k = p[2t+k]:
      V0 = d0-d2, V1 = d1+d2, V2 = d2-d1, V3 = d1-d3      (input xform)
      M_j = sum_dy U_j(dy) @ V_j(row+dy)                   (matmuls)
      out[2t] = M0+M1+M2,  out[2t+1] = M1-M2-M3            (output xform)
    U_j are host-transformed weights: u0=g0, u1=(g0+g1+g2)/2,
    u2=(g0-g1+g2)/2, u3=g2.
  - The 4 V input planes are precomputed on the host (prep is free) and
    DMA'd in directly; no on-device input transform at all.
  - h convs: per gate 4 j-planes x 3 dy = 12 K=128 matmuls per chunk.
  - x convs: the V^x planes pack [rows r ; rows r+1] on 128 partitions,
    so one matmul covers dy=0+1; dy=2 is a K=64 matmul on the lower
    half: 8 x-matmuls per gate per chunk. 20 total (vs 28 direct).
  - Chunk = 16 output rows: M_j is [128, 512] = one full PSUM bank;
    4 banks per gate, 2 gates in flight (8 banks).
  - Output transform on DVE with one-PSUM-operand chains (HW limit):
    even: e1=M1+pe_e; e2=e1+M2; M0=e2+M0; sigmoid reads the M0 bank.
    odd:  o1=M1+pe_o; o2=o1-M2; o3=o2-M3; sigmoid reads SBUF o3.
  - Peephole products (Wc*c) are precomputed on the host in the eo
    layout and DMA'd in; bias rides the activation instruction.
  - og/c_new/h_new are assembled pixel-interleaved in SBUF (strided dst
    APs are free for 1x-rate ops) so output DMAs are contiguous fp32.
  - Matmuls in fp16, accumulate fp32; elementwise fp16 where possible;
    t1/t2/c_new/h_new on GpSimd.
"""

import os
import numpy as np

B, CIN, CHID, H, W, K = 16, 64, 128, 64, 64, 3
N_CORES = 8
PER = B // N_CORES          # images per core
WPAD = W + 2                # 66 padded cols = 33 even + 33 odd
RPAD = H + 2                # 66 padded rows
NT = W // 2                 # 32 tiles per row
NJ = 4                      # winograd planes
VFLAT = RPAD * NT           # 2112 = V plane cols per channel
CHUNK_ROWS = 16
CCT = CHUNK_ROWS * NT       # 512 tile-cols per chunk = one PSUM bank
CCP = CHUNK_ROWS * W        # 1024 pixels per chunk
N_CHUNKS = H // CHUNK_ROWS  # 4
HW = H * W

# gate processing order: candidate first (tanh can start early), output gate
# last (shortest tail after the final matmul). Index meaning: 0=i 1=f 2=o 3=c
GORDER = [3, 0, 1, 2]
# V-plane DMA row pieces: small piece 0 so chunk 0's later j-planes are
# not starved at startup; pieces 1/2 are issued during chunks 0/1
VPIECES = [(0, 18), (18, 34), (34, 66)]
# output row chunks per image: the last image ends with two 8-row chunks so
# the post-matmul tail (combines/sigmoid/h_new/DMA) is halved
CHUNKS_MID = [(0, 16), (16, 16), (32, 16), (48, 16)]
CHUNKS_LAST = [(0, 16), (16, 16), (32, 16), (48, 8), (56, 8)]

_PROG = None
LAST_RESULTS = None


def _build_program():
    import concourse.bacc as bacc
    import concourse.tile as tile
    import concourse.mybir as mybir
    from contextlib import ExitStack

    f32 = mybir.dt.float32
    f16 = mybir.dt.float16

    nc = bacc.Bacc("TRN2", target_bir_lowering=False, debug=False,
                   num_devices=N_CORES)

    vh_d = nc.dram_tensor("vh", [PER, NJ, CHID, VFLAT], f16,
                          kind="ExternalInput").ap()
    vx_d = nc.dram_tensor("vx", [PER, NJ, 2 * CIN, VFLAT], f16,
                          kind="ExternalInput").ap()
    c_d = nc.dram_tensor("c", [PER, CHID, HW], f16, kind="ExternalInput").ap()
    pe_d = nc.dram_tensor("pe", [3, PER, CHID, HW], f16,
                          kind="ExternalInput").ap()
    whw_d = nc.dram_tensor("whw", [4, CHID, 12 * CHID], f16,
                           kind="ExternalInput").ap()
    wxw_d = nc.dram_tensor("wxw", [4, CHID, 4 * CHID], f16,
                           kind="ExternalInput").ap()
    wx2_d = nc.dram_tensor("wx2", [4, CHID, 4 * CHID], f16,
                           kind="ExternalInput").ap()
    bias_d = nc.dram_tensor("bias", [CHID, 4], f32, kind="ExternalInput").ap()
    og_d = nc.dram_tensor("og", [PER, CHID, HW], f32, kind="ExternalOutput").ap()
    hn_d = nc.dram_tensor("hn", [PER, CHID, HW], f32, kind="ExternalOutput").ap()
    cn_d = nc.dram_tensor("cn", [PER, CHID, HW], f32, kind="ExternalOutput").ap()

    SIG = mybir.ActivationFunctionType.Sigmoid
    TANH = mybir.ActivationFunctionType.Tanh

    with tile.TileContext(nc) as tc, ExitStack() as ctx:
        const = ctx.enter_context(tc.tile_pool(name="const", bufs=1))
        vpool = ctx.enter_context(tc.tile_pool(name="vpool", bufs=2))
        work = ctx.enter_context(tc.tile_pool(name="work", bufs=2))
        outs = ctx.enter_context(tc.tile_pool(name="outs", bufs=2))
        psum = ctx.enter_context(tc.tile_pool(name="psum", bufs=8, space="PSUM"))

        # ---- weights on the Activation HWDGE queue, consumption order.
        # Gate 3 (candidate, processed first) is issued before the first
        # image's V^x pieces; the remaining gates after them.
        whw_t, wxw_t, wx2_t = {}, {}, {}

        def walloc(g):
            whw_t[g] = [const.tile([CHID, 6 * CHID], f16, name=f"whw{g}_{p}")
                        for p in range(2)]
            wxw_t[g] = const.tile([CHID, 4 * CHID], f16, name=f"wxw{g}")
            wx2_t[g] = const.tile([CHID, 4 * CHID], f16, name=f"wx2{g}")

        def wdma(g):
            nc.scalar.dma_start(whw_t[g][0][:], whw_d[g][:, 0:6 * CHID])
            nc.scalar.dma_start(wxw_t[g][:], wxw_d[g])
            nc.scalar.dma_start(wx2_t[g][:], wx2_d[g])
            nc.scalar.dma_start(whw_t[g][1][:],
                                whw_d[g][:, 6 * CHID:12 * CHID])

        for g in GORDER:
            walloc(g)
        bias_t = const.tile([CHID, 4], f32)
        wdma(GORDER[0])
        nc.scalar.dma_start(bias_t[:], bias_d)

        def uh(g, j, dy):
            p, blk = divmod(j * 3 + dy, 6)
            return whw_t[g][p][:, blk * CHID:(blk + 1) * CHID]

        for b in range(PER):
            chunks = CHUNKS_LAST if b == PER - 1 else CHUNKS_MID
            # V planes: vh rides the SP queue, vx the Activation queue
            # (interleaved with the weight tiles at startup)
            vh = [vpool.tile([CHID, VFLAT], f16, tag=f"vh{j}",
                             name=f"vh{b}_{j}") for j in range(NJ)]
            vx = [vpool.tile([2 * CIN, VFLAT], f16, tag=f"vx{j}",
                             name=f"vx{b}_{j}") for j in range(NJ)]

            def vdma(piece):
                r0, r1 = VPIECES[piece]
                s, e = r0 * NT, r1 * NT
                for j in range(NJ):
                    nc.sync.dma_start(vh[j][:, s:e], vh_d[b][j][:, s:e])
                    nc.scalar.dma_start(vx[j][:, s:e], vx_d[b][j][:, s:e])

            vdma(0)
            if b == 0:
                for g in GORDER[1:]:
                    wdma(g)

            for kc, (row0, nrows) in enumerate(chunks):
                cct = nrows * NT          # psum bank cols (tile space)
                ccp = nrows * W           # pixels in chunk
                c0 = row0 * W
                last = (b == PER - 1 and kc == len(chunks) - 1)
                ps = {}
                for g in GORDER:
                    m = [psum.tile([CHID, cct], f32, tag="ps",
                                   padded_shape=[CHID, CCT],
                                   name=f"ps{b}_{kc}_{g}_{j}")
                         for j in range(NJ)]
                    ps[g] = m
                    # final gate of the final chunk: do the M0 plane last so
                    # the odd-side chain drains during its matmuls
                    jorder = [1, 2, 3, 0] if (last and g == 2) else range(NJ)
                    for j in jorder:
                        for dy in range(3):
                            nc.tensor.matmul(
                                m[j][:], uh(g, j, dy),
                                vh[j][:, (row0 + dy) * NT:
                                       (row0 + dy) * NT + cct],
                                start=(dy == 0), stop=False)
                        nc.tensor.matmul(
                            m[j][:], wxw_t[g][:, j * CHID:(j + 1) * CHID],
                            vx[j][:, row0 * NT:row0 * NT + cct],
                            start=False, stop=False)
                        # dy=2: zero lower weight rows; the packed upper
                        # partitions at col offset row0+1 hold V^x(row+2)
                        nc.tensor.matmul(
                            m[j][:], wx2_t[g][:, j * CHID:(j + 1) * CHID],
                            vx[j][:, (row0 + 1) * NT:(row0 + 1) * NT + cct],
                            start=False, stop=True)

                # c and peephole chunks (host-precomputed pe = Wc*c), eo layout
                ctc = outs.tile([CHID, CCP], f16, tag="ct", bufs=3,
                                name=f"ct{b}_{kc}")
                nc.sync.dma_start(ctc[:, 0:ccp], c_d[b][:, c0:c0 + ccp])
                pec = []
                for gi in range(3):
                    t = outs.tile([CHID, CCP], f16, tag=f"pe{gi}", bufs=3,
                                  name=f"pe{b}_{kc}_{gi}")
                    nc.sync.dma_start(t[:, 0:ccp], pe_d[gi][b][:, c0:c0 + ccp])
                    pec.append(t)
                if kc == 0:
                    vdma(1)
                elif kc == 1:
                    vdma(2)

                def v64(t):  # [128, ccp] -> [128, rows, 64]
                    return t[:, 0:ccp].rearrange("p (r s) -> p r s", s=64)

                def vt(pb):  # psum bank [128, cct] -> [128, rows, 32]
                    return pb[:].rearrange("p (r t) -> p r t", t=NT)

                # activations / gate results, [row][even 32 | odd 32] layout
                gc = work.tile([CHID, CCP], f16, tag="gc")
                ig = work.tile([CHID, CCP], f16, tag="ig")
                fg = work.tile([CHID, CCP], f16, tag="fg")
                # og/cn/hn assembled pixel-interleaved for contiguous DMA
                ogt = outs.tile([CHID, CCP], f32, tag="og")
                cnt = outs.tile([CHID, CCP], f32, tag="cn")
                hnt = outs.tile([CHID, CCP], f32, tag="hn")
                ogi = ogt[:, 0:ccp].rearrange("p (r t eo) -> p r t eo",
                                              eo=2, t=NT)

                for g in GORDER:
                    m = ps[g]
                    gi = g  # 0=i 1=f 2=o 3=c
                    if gi == 3:
                        # candidate: copy M1 on DVE (keeps the Act queue
                        # free of copies the DVE chain would wait on)
                        m1c = work.tile([CHID, CCT], f32, tag="m1c")
                        nc.vector.tensor_scalar_add(m1c[:, 0:cct], m[1][:],
                                                    0.0)
                        e1 = work.tile([CHID, CCT], f32, tag="ce1")
                        nc.vector.tensor_add(e1[:, 0:cct], m1c[:, 0:cct],
                                             m[2][:])
                        nc.vector.tensor_add(m[0][:], e1[:, 0:cct], m[0][:])
                        o1 = work.tile([CHID, CCT], f32, tag="co1")
                        nc.vector.tensor_sub(o1[:, 0:cct], m1c[:, 0:cct],
                                             m[2][:])
                        o2 = work.tile([CHID, CCT], f32, tag="co2")
                        nc.vector.tensor_sub(o2[:, 0:cct], o1[:, 0:cct],
                                             m[3][:])
                        nc.scalar.activation(v64(gc)[:, :, 0:NT], vt(m[0]),
                                             TANH, bias=bias_t[:, 3:4])
                        nc.scalar.activation(v64(gc)[:, :, NT:W],
                                             o2[:, 0:cct].rearrange(
                                                 "p (r t) -> p r t", t=NT),
                                             TANH, bias=bias_t[:, 3:4])
                        continue
                    pv = v64(pec[gi])
                    # even = M0+M1+M2+pe_e, finishing in the M0 bank
                    e1 = work.tile([CHID, CCT], f32, tag="e1",
                                   name=f"e1_{b}_{kc}_{gi}")
                    nc.vector.tensor_add(vt(e1[:, 0:cct]), pv[:, :, 0:NT],
                                         vt(m[1]))
                    e2 = work.tile([CHID, CCT], f32, tag="e2",
                                   name=f"e2_{b}_{kc}_{gi}")
                    nc.vector.tensor_add(e2[:, 0:cct], e1[:, 0:cct],
                                         m[2][:])
                    nc.vector.tensor_add(m[0][:], e2[:, 0:cct], m[0][:])
                    # odd = M1-M2-M3+pe_o in SBUF
                    o1 = work.tile([CHID, CCT], f32, tag="o1",
                                   name=f"o1_{b}_{kc}_{gi}")
                    nc.vector.tensor_add(vt(o1[:, 0:cct]), pv[:, :, NT:W],
                                         vt(m[1]))
                    o2 = work.tile([CHID, CCT], f32, tag="o2",
                                   name=f"o2_{b}_{kc}_{gi}")
                    nc.vector.tensor_sub(o2[:, 0:cct], o1[:, 0:cct],
                                         m[2][:])
                    o3 = work.tile([CHID, CCT], f32, tag="o3",
                                   name=f"o3_{b}_{kc}_{gi}")
                    nc.vector.tensor_sub(o3[:, 0:cct], o2[:, 0:cct],
                                         m[3][:])
                    if gi == 0:
                        dste, dsto = v64(ig)[:, :, 0:NT], v64(ig)[:, :, NT:W]
                    elif gi == 1:
                        dste, dsto = v64(fg)[:, :, 0:NT], v64(fg)[:, :, NT:W]
                    else:
                        dste, dsto = ogi[:, :, :, 0], ogi[:, :, :, 1]
                    nc.scalar.activation(dste, vt(m[0]), SIG,
                                         bias=bias_t[:, gi:gi + 1])
                    nc.scalar.activation(
                        dsto, o3[:, 0:cct].rearrange("p (r t) -> p r t",
                                                     t=NT),
                        SIG, bias=bias_t[:, gi:gi + 1])

                t2 = work.tile([CHID, CCP], f16, tag="t2")
                nc.gpsimd.tensor_mul(t2[:, 0:ccp], ig[:, 0:ccp],
                                     gc[:, 0:ccp])
                t1 = work.tile([CHID, CCP], f16, tag="t1")
                nc.gpsimd.tensor_mul(t1[:, 0:ccp], fg[:, 0:ccp],
                                     ctc[:, 0:ccp])
                cni = cnt[:, 0:ccp].rearrange("p (r t eo) -> p r eo t",
                                              eo=2, t=NT)
                src_eo = lambda t: t[:, 0:ccp].rearrange(
                    "p (r eo t) -> p r eo t", eo=2, t=NT)
                nc.gpsimd.tensor_add(cni, src_eo(t1), src_eo(t2))

                sl = slice(c0, c0 + ccp)
                nc.scalar.dma_start(cn_d[b][:, sl], cnt[:, 0:ccp])
                # h_new split even/odd so the odd half rides right after
                # the last sigmoid; on DVE for the final chunk (shorter tail)
                heng = nc.vector if last else nc.gpsimd
                hni = hnt[:, 0:ccp].rearrange("p (r t eo) -> p r t eo",
                                              eo=2, t=NT)
                cnx = cnt[:, 0:ccp].rearrange("p (r t eo) -> p r t eo",
                                              eo=2, t=NT)
                heng.tensor_mul(hni[:, :, :, 0], ogi[:, :, :, 0],
                                cnx[:, :, :, 0])
                heng.tensor_mul(hni[:, :, :, 1], ogi[:, :, :, 1],
                                cnx[:, :, :, 1])
                nc.sync.dma_start(og_d[b][:, sl], ogt[:, 0:ccp])
                nc.scalar.dma_start(hn_d[b][:, sl], hnt[:, 0:ccp])

    nc.compile()
    return nc


def _pad_eo(a, rowshift=0):
    """[N,C,H,W] fp32 -> even/odd col-split padded fp32 [N,C,RPAD,2,33]."""
    n, ch = a.shape[:2]
    p = np.zeros((n, ch, RPAD + rowshift, WPAD), np.float32)
    p[:, :, 1:H + 1, 1:W + 1] = a
    p = p[:, :, rowshift:rowshift + RPAD]
    return np.stack([p[:, :, :, 0::2], p[:, :, :, 1::2]], axis=3)


def _v_planes(eo):
    """eo: [N,C,RPAD,2,33] -> [N, 4, C, RPAD*NT] fp16 Winograd planes."""
    he, ho = eo[:, :, :, 0], eo[:, :, :, 1]
    v = np.stack([he[..., 0:NT] - he[..., 1:NT + 1],
                  ho[..., 0:NT] + he[..., 1:NT + 1],
                  he[..., 1:NT + 1] - ho[..., 0:NT],
                  ho[..., 0:NT] - ho[..., 1:NT + 1]], axis=1)
    n, _, ch = v.shape[:3]
    return np.ascontiguousarray(v.reshape(n, NJ, ch, RPAD * NT)
                                ).astype(np.float16)


def _eo_pixels(a):
    """[N,C,H,W] fp32 -> [N,C,HW] fp16 with each row as [even 32 | odd 32]."""
    eo = np.concatenate([a[:, :, :, 0::2], a[:, :, :, 1::2]], axis=3)
    return np.ascontiguousarray(eo.reshape(a.shape[0], a.shape[1], HW)
                                ).astype(np.float16)


def _wino_u(g):
    """g: [..., 3] -> [..., 4] Winograd F(2,3) weight transform."""
    u0 = g[..., 0]
    u1 = 0.5 * (g[..., 0] + g[..., 1] + g[..., 2])
    u2 = 0.5 * (g[..., 0] - g[..., 1] + g[..., 2])
    u3 = g[..., 2]
    return np.stack([u0, u1, u2, u3], axis=-1)


def kernel(x, h, c, Wxi, bxi, Whi, Wci, Wxf, bxf, Whf, Wcf,
           Wxo, bxo, Who, Wco, Wxc, bxc, Whc):
    global _PROG, LAST_RESULTS
    from concourse.bass_utils import run_bass_kernel_spmd

    x = np.asarray(x, dtype=np.float32)
    h = np.asarray(h, dtype=np.float32)
    c = np.asarray(c, dtype=np.float32)

    vh = _v_planes(_pad_eo(h))
    # x: [x_pad rows r ; x_pad rows r+1] packed on the channel axis
    xeo = np.concatenate([_pad_eo(x), _pad_eo(x, rowshift=1)], axis=1)
    vx = _v_planes(xeo)
    cf = _eo_pixels(c)
    pe = np.stack([_eo_pixels(np.asarray(wc, np.float32)[None] * c)
                   for wc in (Wci, Wcf, Wco)])  # [3, B, 128, HW] fp16

    def wh_prep(w):
        # [Co=128, Ci=128, 3, 3] -> U [Ci, (j*3+dy)*128 + Co]
        u = _wino_u(np.asarray(w, np.float32))  # [co, ci, dy, j]
        out = np.empty((CHID, 12 * CHID), np.float32)
        for j in range(NJ):
            for dy in range(3):
                out[:, (j * 3 + dy) * CHID:(j * 3 + dy + 1) * CHID] = \
                    u[:, :, dy, j].T
        return out.astype(np.float16)

    def wx_prep(w):
        # [Co=128, Ci=64, 3, 3] -> (packed dy0/dy1 [128, 4*128],
        # dy2 [128, 4*128] with zero lower rows — rhs is the packed vx at
        # row offset +1, whose upper partitions hold V^x(row+2))
        u = _wino_u(np.asarray(w, np.float32))  # [co, ci, dy, j]
        p01 = np.empty((CHID, 4 * CHID), np.float32)
        p2 = np.zeros((CHID, 4 * CHID), np.float32)
        for j in range(NJ):
            p01[:CIN, j * CHID:(j + 1) * CHID] = u[:, :, 0, j].T
            p01[CIN:, j * CHID:(j + 1) * CHID] = u[:, :, 1, j].T
            p2[CIN:, j * CHID:(j + 1) * CHID] = u[:, :, 2, j].T
        return p01.astype(np.float16), p2.astype(np.float16)

    whw = np.stack([wh_prep(w) for w in (Whi, Whf, Who, Whc)])
    wxp = [wx_prep(w) for w in (Wxi, Wxf, Wxo, Wxc)]
    wxw = np.stack([p for p, _ in wxp])
    wx2 = np.stack([q for _, q in wxp])
    bias = np.ascontiguousarray(np.stack(
        [np.asarray(v, dtype=np.float32) for v in (bxi, bxf, bxo, bxc)], axis=1))

    if _PROG is None:
        _PROG = _build_program()

    in_maps = []
    for i in range(N_CORES):
        sl = slice(i * PER, (i + 1) * PER)
        in_maps.append({
            "vh": np.ascontiguousarray(vh[sl]),
            "vx": np.ascontiguousarray(vx[sl]),
            "c": np.ascontiguousarray(cf[sl]),
            "pe": np.ascontiguousarray(pe[:, sl]),
            "whw": whw, "wxw": wxw, "wx2": wx2, "bias": bias,
        })

    res = run_bass_kernel_spmd(nc=_PROG, in_maps=in_maps,
                               core_ids=list(range(N_CORES)),
                               trace=bool(os.environ.get("KERNEL_TRACE")))
    LAST_RESULTS = res

    og = np.empty((B, CHID, HW), dtype=np.float32)
    hn = np.empty((B, CHID, HW), dtype=np.float32)
    cn = np.empty((B, CHID, HW), dtype=np.float32)
    for i in range(N_CORES):
        sl = slice(i * PER, (i + 1) * PER)
        og[sl] = res.results[i]["og"]
        hn[sl] = res.results[i]["hn"]
        cn[sl] = res.results[i]["cn"]

    shape = (B, CHID, H, W)
    return (og.reshape(shape), hn.reshape(shape), cn.reshape(shape))


# revision 34
# speedup vs baseline: 1.1772x; 1.0569x over previous
"""ConvLSTM cell forward on 8 Trainium2 NeuronCores — Winograd F(2,3).

Problem: B=16, Cin=64, Chid=128, H=W=64, K=3 (SAME padding).
  ig = sigmoid(conv(x,Wxi) + bxi + conv(h,Whi) + Wci*c)
  fg = sigmoid(conv(x,Wxf) + bxf + conv(h,Whf) + Wcf*c)
  c_new = fg*c + ig*tanh(conv(x,Wxc) + bxc + conv(h,Whc))
  og = sigmoid(conv(x,Wxo) + bxo + conv(h,Who) + Wco*c)
  h_new = og*c_new
  returns (og, h_new, c_new)

Strategy:
  - Data-parallel over batch: 2 images per core, weights replicated.
  - Winograd F(2,3) along W cuts the 3-tap x-dim contraction from 3
    multiplies per output to 4 per 2 outputs. Per output-column pair
    (2t, 2t+1) with padded input cols d_k = p[2t+k]:
      V0 = d0-d2, V1 = d1+d2, V2 = d2-d1, V3 = d1-d3      (input xform)
      M_j = sum_dy U_j(dy) @ V_j(row+dy)                   (matmuls)
      out[2t] = M0+M1+M2,  out[2t+1] = M1-M2-M3            (output xform)
    U_j are host-transformed weights: u0=g0, u1=(g0+g1+g2)/2,
    u2=(g0-g1+g2)/2, u3=g2.
  - The 4 V input planes are precomputed on the host (prep is free) and
    DMA'd in directly; no on-device input transform at all.
  - h convs: per gate 4 j-planes x 3 dy = 12 K=128 matmuls per chunk.
  - x convs: the V^x planes pack [rows r ; rows r+1] on 128 partitions,
    so one matmul covers dy=0+1; dy=2 is a K=64 matmul on the lower
    half: 8 x-matmuls per gate per chunk. 20 total (vs 28 direct).
  - Chunk = 16 output rows: M_j is [128, 512] = one full PSUM bank;
    4 banks per gate, 2 gates in flight (8 banks).
  - Output transform on DVE with one-PSUM-operand chains (HW limit):
    even: e1=M1+pe_e; e2=e1+M2; M0=e2+M0; sigmoid reads the M0 bank.
    odd:  o1=M1+pe_o; o2=o1-M2; o3=o2-M3; sigmoid reads SBUF o3.
  - Peephole products (Wc*c) are precomputed on the host in the eo
    layout and DMA'd in; bias rides the activation instruction.
  - og/c_new/h_new are assembled pixel-interleaved in SBUF (strided dst
    APs are free for 1x-rate ops) so output DMAs are contiguous fp32.
  - Matmuls in fp16, accumulate fp32; elementwise fp16 where possible;
    t1/t2/c_new/h_new on GpSimd.
"""

import os
import numpy as np

B, CIN, CHID, H, W, K = 16, 64, 128, 64, 64, 3
N_CORES = 8
PER = B // N_CORES          # images per core
WPAD = W + 2                # 66 padded cols = 33 even + 33 odd
RPAD = H + 2                # 66 padded rows
NT = W // 2                 # 32 tiles per row
NJ = 4                      # winograd planes
VFLAT = RPAD * NT           # 2112 = V plane cols per channel
CHUNK_ROWS = 16
CCT = CHUNK_ROWS * NT       # 512 tile-cols per chunk = one PSUM bank
CCP = CHUNK_ROWS * W        # 1024 pixels per chunk
N_CHUNKS = H // CHUNK_ROWS  # 4
HW = H * W

# gate processing order: candidate first (tanh can start early), output gate
# last (shortest tail after the final matmul). Index meaning: 0=i 1=f 2=o 3=c
GORDER = [3, 0, 1, 2]
# V-plane DMA row pieces: small piece 0 so chunk 0's later j-planes are
# not starved at startup; pieces 1/2 are issued during chunks 0/1
VPIECES = [(0, 18), (18, 34), (34, 66)]
# output row chunks per image: the last image ends with two 8-row chunks so
# the post-matmul tail (combines/sigmoid/h_new/DMA) is halved
CHUNKS_MID = [(0, 16), (16, 16), (32, 16), (48, 16)]
CHUNKS_LAST = [(0, 16), (16, 16), (32, 16), (48, 8), (56, 8)]

_PROG = None
LAST_RESULTS = None


def _build_program():
    import concourse.bacc as bacc
    import concourse.tile as tile
    import concourse.mybir as mybir
    from contextlib import ExitStack

    f32 = mybir.dt.float32
    f16 = mybir.dt.float16

    nc = bacc.Bacc("TRN2", target_bir_lowering=False, debug=False,
                   num_devices=N_CORES)

    vh_d = nc.dram_tensor("vh", [PER, NJ, CHID, VFLAT], f16,
                          kind="ExternalInput").ap()
    vx_d = nc.dram_tensor("vx", [PER, NJ, 2 * CIN, VFLAT], f16,
                          kind="ExternalInput").ap()
    c_d = nc.dram_tensor("c", [PER, CHID, HW], f16, kind="ExternalInput").ap()
    pe_d = nc.dram_tensor("pe", [3, PER, CHID, HW], f16,
                          kind="ExternalInput").ap()
    whw_d = nc.dram_tensor("whw", [4, CHID, 12 * CHID], f16,
                           kind="ExternalInput").ap()
    wxw_d = nc.dram_tensor("wxw", [4, CHID, 4 * CHID], f16,
                           kind="ExternalInput").ap()
    wx2_d = nc.dram_tensor("wx2", [4, CHID, 4 * CHID], f16,
                           kind="ExternalInput").ap()
    bias_d = nc.dram_tensor("bias", [CHID, 4], f32, kind="ExternalInput").ap()
    og_d = nc.dram_tensor("og", [PER, CHID, HW], f32, kind="ExternalOutput").ap()
    hn_d = nc.dram_tensor("hn", [PER, CHID, HW], f32, kind="ExternalOutput").ap()
    cn_d = nc.dram_tensor("cn", [PER, CHID, HW], f32, kind="ExternalOutput").ap()

    SIG = mybir.ActivationFunctionType.Sigmoid
    TANH = mybir.ActivationFunctionType.Tanh

    with tile.TileContext(nc) as tc, ExitStack() as ctx:
        const = ctx.enter_context(tc.tile_pool(name="const", bufs=1))
        vpool = ctx.enter_context(tc.tile_pool(name="vpool", bufs=2))
        work = ctx.enter_context(tc.tile_pool(name="work", bufs=2))
        outs = ctx.enter_context(tc.tile_pool(name="outs", bufs=2))
        psum = ctx.enter_context(tc.tile_pool(name="psum", bufs=8, space="PSUM"))

        # ---- weights on the Activation HWDGE queue, consumption order.
        # Gate 3 (candidate, processed first) is issued before the first
        # image's V^x pieces; the remaining gates after them.
        whw_t, wxw_t, wx2_t = {}, {}, {}

        def walloc(g):
            whw_t[g] = [const.tile([CHID, 6 * CHID], f16, name=f"whw{g}_{p}")
                        for p in range(2)]
            wxw_t[g] = const.tile([CHID, 4 * CHID], f16, name=f"wxw{g}")
            wx2_t[g] = const.tile([CHID, 4 * CHID], f16, name=f"wx2{g}")

        def wdma(g):
            nc.scalar.dma_start(whw_t[g][0][:], whw_d[g][:, 0:6 * CHID])
            nc.scalar.dma_start(wxw_t[g][:], wxw_d[g])
            nc.scalar.dma_start(wx2_t[g][:], wx2_d[g])
            nc.scalar.dma_start(whw_t[g][1][:],
                                whw_d[g][:, 6 * CHID:12 * CHID])

        for g in GORDER:
            walloc(g)
        bias_t = const.tile([CHID, 4], f32)
        wdma(GORDER[0])
        nc.scalar.dma_start(bias_t[:], bias_d)

        def uh(g, j, dy):
            p, blk = divmod(j * 3 + dy, 6)
            return whw_t[g][p][:, blk * CHID:(blk + 1) * CHID]

        for b in range(PER):
            chunks = CHUNKS_LAST if b == PER - 1 else CHUNKS_MID
            # V planes: vh rides the SP queue, vx the Activation queue
            # (interleaved with the weight tiles at startup)
            vh = [vpool.tile([CHID, VFLAT], f16, tag=f"vh{j}",
                             name=f"vh{b}_{j}") for j in range(NJ)]
            vx = [vpool.tile([2 * CIN, VFLAT], f16, tag=f"vx{j}",
                             name=f"vx{b}_{j}") for j in range(NJ)]

            def vdma(piece):
                r0, r1 = VPIECES[piece]
                s, e = r0 * NT, r1 * NT
                # image 0's vx pieces interleave with the weight tiles on
                # the Activation queue (cold start, before any sigmoids);
                # image 1 keeps the Activation queue clean mid-stream
                vxq = nc.scalar if b == 0 else nc.sync
                for j in range(NJ):
                    nc.sync.dma_start(vh[j][:, s:e], vh_d[b][j][:, s:e])
                    vxq.dma_start(vx[j][:, s:e], vx_d[b][j][:, s:e])

            vdma(0)
            if b == 0:
                for g in GORDER[1:]:
                    wdma(g)

            for kc, (row0, nrows) in enumerate(chunks):
                cct = nrows * NT          # psum bank cols (tile space)
                ccp = nrows * W           # pixels in chunk
                c0 = row0 * W
                last = (b == PER - 1 and kc == len(chunks) - 1)
                ps = {}
                for g in GORDER:
                    m = [psum.tile([CHID, cct], f32, tag="ps",
                                   padded_shape=[CHID, CCT],
                                   name=f"ps{b}_{kc}_{g}_{j}")
                         for j in range(NJ)]
                    ps[g] = m
                    # final gate of the final chunk: do the M0 plane last so
                    # the odd-side chain drains during its matmuls
                    jorder = [1, 2, 3, 0] if (last and g == 2) else range(NJ)
                    for j in jorder:
                        for dy in range(3):
                            nc.tensor.matmul(
                                m[j][:], uh(g, j, dy),
                                vh[j][:, (row0 + dy) * NT:
                                       (row0 + dy) * NT + cct],
                                start=(dy == 0), stop=False)
                        nc.tensor.matmul(
                            m[j][:], wxw_t[g][:, j * CHID:(j + 1) * CHID],
                            vx[j][:, row0 * NT:row0 * NT + cct],
                            start=False, stop=False)
                        # dy=2: zero lower weight rows; the packed upper
                        # partitions at col offset row0+1 hold V^x(row+2)
                        nc.tensor.matmul(
                            m[j][:], wx2_t[g][:, j * CHID:(j + 1) * CHID],
                            vx[j][:, (row0 + 1) * NT:(row0 + 1) * NT + cct],
                            start=False, stop=True)

                # c and peephole chunks (host-precomputed pe = Wc*c), eo layout
                ctc = outs.tile([CHID, CCP], f16, tag="ct", bufs=3,
                                name=f"ct{b}_{kc}")
                nc.sync.dma_start(ctc[:, 0:ccp], c_d[b][:, c0:c0 + ccp])
                pec = []
                for gi in range(3):
                    t = outs.tile([CHID, CCP], f16, tag=f"pe{gi}", bufs=3,
                                  name=f"pe{b}_{kc}_{gi}")
                    nc.sync.dma_start(t[:, 0:ccp], pe_d[gi][b][:, c0:c0 + ccp])
                    pec.append(t)
                if kc == 0:
                    vdma(1)
                elif kc == 1:
                    vdma(2)

                def v64(t):  # [128, ccp] -> [128, rows, 64]
                    return t[:, 0:ccp].rearrange("p (r s) -> p r s", s=64)

                def vt(pb):  # psum bank [128, cct] -> [128, rows, 32]
                    return pb[:].rearrange("p (r t) -> p r t", t=NT)

                # activations / gate results, [row][even 32 | odd 32] layout
                gc = work.tile([CHID, CCP], f16, tag="gc")
                ig = work.tile([CHID, CCP], f16, tag="ig")
                fg = work.tile([CHID, CCP], f16, tag="fg")
                # og/cn/hn assembled pixel-interleaved for contiguous DMA
                ogt = outs.tile([CHID, CCP], f32, tag="og")
                cnt = outs.tile([CHID, CCP], f32, tag="cn")
                hnt = outs.tile([CHID, CCP], f32, tag="hn")
                ogi = ogt[:, 0:ccp].rearrange("p (r t eo) -> p r t eo",
                                              eo=2, t=NT)

                for g in GORDER:
                    m = ps[g]
                    gi = g  # 0=i 1=f 2=o 3=c
                    if gi == 3:
                        # candidate: copy M1 on DVE (keeps the Act queue
                        # free of copies the DVE chain would wait on)
                        m1c = work.tile([CHID, CCT], f32, tag="m1c")
                        nc.vector.tensor_scalar_add(m1c[:, 0:cct], m[1][:],
                                                    0.0)
                        e1 = work.tile([CHID, CCT], f32, tag="ce1")
                        nc.vector.tensor_add(e1[:, 0:cct], m1c[:, 0:cct],
                                             m[2][:])
                        nc.vector.tensor_add(m[0][:], e1[:, 0:cct], m[0][:])
                        o1 = work.tile([CHID, CCT], f32, tag="co1")
                        nc.vector.tensor_sub(o1[:, 0:cct], m1c[:, 0:cct],
                                             m[2][:])
                        o2 = work.tile([CHID, CCT], f32, tag="co2")
                        nc.vector.tensor_sub(o2[:, 0:cct], o1[:, 0:cct],
                                             m[3][:])
                        nc.scalar.activation(v64(gc)[:, :, 0:NT], vt(m[0]),
                                             TANH, bias=bias_t[:, 3:4])
                        nc.scalar.activation(v64(gc)[:, :, NT:W],
                                             o2[:, 0:cct].rearrange(
                                                 "p (r t) -> p r t", t=NT),
                                             TANH, bias=bias_t[:, 3:4])
                        continue
                    pv = v64(pec[gi])
                    # even = M0+M1+M2+pe_e, finishing in the M0 bank
                    e1 = work.tile([CHID, CCT], f32, tag="e1",
                                   name=f"e1_{b}_{kc}_{gi}")
                    nc.vector.tensor_add(vt(e1[:, 0:cct]), pv[:, :, 0:NT],
                                         vt(m[1]))
                    e2 = work.tile([CHID, CCT], f32, tag="e2",
                                   name=f"e2_{b}_{kc}_{gi}")
                    nc.vector.tensor_add(e2[:, 0:cct], e1[:, 0:cct],
                                         m[2][:])
                    nc.vector.tensor_add(m[0][:], e2[:, 0:cct], m[0][:])
                    # odd = M1-M2-M3+pe_o in SBUF
                    o1 = work.tile([CHID, CCT], f32, tag="o1",
                                   name=f"o1_{b}_{kc}_{gi}")
                    nc.vector.tensor_add(vt(o1[:, 0:cct]), pv[:, :, NT:W],
                                         vt(m[1]))
                    o2 = work.tile([CHID, CCT], f32, tag="o2",
                                   name=f"o2_{b}_{kc}_{gi}")
                    nc.vector.tensor_sub(o2[:, 0:cct], o1[:, 0:cct],
                                         m[2][:])
                    o3 = work.tile([CHID, CCT], f32, tag="o3",
                                   name=f"o3_{b}_{kc}_{gi}")
                    nc.vector.tensor_sub(o3[:, 0:cct], o2[:, 0:cct],
                                         m[3][:])
                    if gi == 0:
                        dste, dsto = v64(ig)[:, :, 0:NT], v64(ig)[:, :, NT:W]
                    elif gi == 1:
                        dste, dsto = v64(fg)[:, :, 0:NT], v64(fg)[:, :, NT:W]
                    else:
                        dste, dsto = ogi[:, :, :, 0], ogi[:, :, :, 1]
                    nc.scalar.activation(dste, vt(m[0]), SIG,
                                         bias=bias_t[:, gi:gi + 1])
                    nc.scalar.activation(
                        dsto, o3[:, 0:cct].rearrange("p (r t) -> p r t",
                                                     t=NT),
                        SIG, bias=bias_t[:, gi:gi + 1])

                t2 = work.tile([CHID, CCP], f16, tag="t2")
                nc.gpsimd.tensor_mul(t2[:, 0:ccp], ig[:, 0:ccp],
                                     gc[:, 0:ccp])
                t1 = work.tile([CHID, CCP], f16, tag="t1")
                nc.gpsimd.tensor_mul(t1[:, 0:ccp], fg[:, 0:ccp],
                                     ctc[:, 0:ccp])
                cni = cnt[:, 0:ccp].rearrange("p (r t eo) -> p r eo t",
                                              eo=2, t=NT)
                src_eo = lambda t: t[:, 0:ccp].rearrange(
                    "p (r eo t) -> p r eo t", eo=2, t=NT)
                nc.gpsimd.tensor_add(cni, src_eo(t1), src_eo(t2))

                sl = slice(c0, c0 + ccp)
                nc.sync.dma_start(cn_d[b][:, sl], cnt[:, 0:ccp])
                # h_new split even/odd so the odd half rides right after
                # the last sigmoid; on DVE for the final chunk (shorter tail)
                heng = nc.vector if last else nc.gpsimd
                hni = hnt[:, 0:ccp].rearrange("p (r t eo) -> p r t eo",
                                              eo=2, t=NT)
                cnx = cnt[:, 0:ccp].rearrange("p (r t eo) -> p r t eo",
                                              eo=2, t=NT)
                heng.tensor_mul(hni[:, :, :, 0], ogi[:, :, :, 0],
                                cnx[:, :, :, 0])
                heng.tensor_mul(hni[:, :, :, 1], ogi[:, :, :, 1],
                                cnx[:, :, :, 1])
                nc.sync.dma_start(og_d[b][:, sl], ogt[:, 0:ccp])
                nc.sync.dma_start(hn_d[b][:, sl], hnt[:, 0:ccp])

    nc.compile()
    return nc


def _pad_eo(a, rowshift=0):
    """[N,C,H,W] fp32 -> even/odd col-split padded fp32 [N,C,RPAD,2,33]."""
    n, ch = a.shape[:2]
    p = np.zeros((n, ch, RPAD + rowshift, WPAD), np.float32)
    p[:, :, 1:H + 1, 1:W + 1] = a
    p = p[:, :, rowshift:rowshift + RPAD]
    return np.stack([p[:, :, :, 0::2], p[:, :, :, 1::2]], axis=3)


def _v_planes(eo):
    """eo: [N,C,RPAD,2,33] -> [N, 4, C, RPAD*NT] fp16 Winograd planes."""
    he, ho = eo[:, :, :, 0], eo[:, :, :, 1]
    v = np.stack([he[..., 0:NT] - he[..., 1:NT + 1],
                  ho[..., 0:NT] + he[..., 1:NT + 1],
                  he[..., 1:NT + 1] - ho[..., 0:NT],
                  ho[..., 0:NT] - ho[..., 1:NT + 1]], axis=1)
    n, _, ch = v.shape[:3]
    return np.ascontiguousarray(v.reshape(n, NJ, ch, RPAD * NT)
                                ).astype(np.float16)


def _eo_pixels(a):
    """[N,C,H,W] fp32 -> [N,C,HW] fp16 with each row as [even 32 | odd 32]."""
    eo = np.concatenate([a[:, :, :, 0::2], a[:, :, :, 1::2]], axis=3)
    return np.ascontiguousarray(eo.reshape(a.shape[0], a.shape[1], HW)
                                ).astype(np.float16)


def _wino_u(g):
    """g: [..., 3] -> [..., 4] Winograd F(2,3) weight transform."""
    u0 = g[..., 0]
    u1 = 0.5 * (g[..., 0] + g[..., 1] + g[..., 2])
    u2 = 0.5 * (g[..., 0] - g[..., 1] + g[..., 2])
    u3 = g[..., 2]
    return np.stack([u0, u1, u2, u3], axis=-1)


def kernel(x, h, c, Wxi, bxi, Whi, Wci, Wxf, bxf, Whf, Wcf,
           Wxo, bxo, Who, Wco, Wxc, bxc, Whc):
    global _PROG, LAST_RESULTS
    from concourse.bass_utils import run_bass_kernel_spmd

    x = np.asarray(x, dtype=np.float32)
    h = np.asarray(h, dtype=np.float32)
    c = np.asarray(c, dtype=np.float32)

    vh = _v_planes(_pad_eo(h))
    # x: [x_pad rows r ; x_pad rows r+1] packed on the channel axis
    xeo = np.concatenate([_pad_eo(x), _pad_eo(x, rowshift=1)], axis=1)
    vx = _v_planes(xeo)
    cf = _eo_pixels(c)
    pe = np.stack([_eo_pixels(np.asarray(wc, np.float32)[None] * c)
                   for wc in (Wci, Wcf, Wco)])  # [3, B, 128, HW] fp16

    def wh_prep(w):
        # [Co=128, Ci=128, 3, 3] -> U [Ci, (j*3+dy)*128 + Co]
        u = _wino_u(np.asarray(w, np.float32))  # [co, ci, dy, j]
        out = np.empty((CHID, 12 * CHID), np.float32)
        for j in range(NJ):
            for dy in range(3):
                out[:, (j * 3 + dy) * CHID:(j * 3 + dy + 1) * CHID] = \
                    u[:, :, dy, j].T
        return out.astype(np.float16)

    def wx_prep(w):
        # [Co=128, Ci=64, 3, 3] -> (packed dy0/dy1 [128, 4*128],
        # dy2 [128, 4*128] with zero lower rows — rhs is the packed vx at
        # row offset +1, whose upper partitions hold V^x(row+2))
        u = _wino_u(np.asarray(w, np.float32))  # [co, ci, dy, j]
        p01 = np.empty((CHID, 4 * CHID), np.float32)
        p2 = np.zeros((CHID, 4 * CHID), np.float32)
        for j in range(NJ):
            p01[:CIN, j * CHID:(j + 1) * CHID] = u[:, :, 0, j].T
            p01[CIN:, j * CHID:(j + 1) * CHID] = u[:, :, 1, j].T
            p2[CIN:, j * CHID:(j + 1) * CHID] = u[:, :, 2, j].T
        return p01.astype(np.float16), p2.astype(np.float16)

    whw = np.stack([wh_prep(w) for w in (Whi, Whf, Who, Whc)])
    wxp = [wx_prep(w) for w in (Wxi, Wxf, Wxo, Wxc)]
    wxw = np.stack([p for p, _ in wxp])
    wx2 = np.stack([q for _, q in wxp])
    bias = np.ascontiguousarray(np.stack(
        [np.asarray(v, dtype=np.float32) for v in (bxi, bxf, bxo, bxc)], axis=1))

    if _PROG is None:
        _PROG = _build_program()

    in_maps = []
    for i in range(N_CORES):
        sl = slice(i * PER, (i + 1) * PER)
        in_maps.append({
            "vh": np.ascontiguousarray(vh[sl]),
            "vx": np.ascontiguousarray(vx[sl]),
            "c": np.ascontiguousarray(cf[sl]),
            "pe": np.ascontiguousarray(pe[:, sl]),
            "whw": whw, "wxw": wxw, "wx2": wx2, "bias": bias,
        })

    res = run_bass_kernel_spmd(nc=_PROG, in_maps=in_maps,
                               core_ids=list(range(N_CORES)),
                               trace=bool(os.environ.get("KERNEL_TRACE")))
    LAST_RESULTS = res

    og = np.empty((B, CHID, HW), dtype=np.float32)
    hn = np.empty((B, CHID, HW), dtype=np.float32)
    cn = np.empty((B, CHID, HW), dtype=np.float32)
    for i in range(N_CORES):
        sl = slice(i * PER, (i + 1) * PER)
        og[sl] = res.results[i]["og"]
        hn[sl] = res.results[i]["hn"]
        cn[sl] = res.results[i]["cn"]

    shape = (B, CHID, H, W)
    return (og.reshape(shape), hn.reshape(shape), cn.reshape(shape))
